# revision 1
# baseline (speedup 1.0000x reference)
"""ContrastiveHead loss kernel for 8 Trainium2 NeuronCores.

Strategy (per sharding hint): data-parallel shard B across the 8 cores.
Each core runs the 3-layer MLP for its 2*B/8 = 1024 rows (input1 and
input2 shards stacked), normalizes the [1024, 128] features, all-gathers
the normalized features (bf16) across cores, then computes its local
[1024, 8192] block of the similarity matrix and the masked logsumexp.

Layouts: activations ride transposed ([features-on-partitions, rows-on-
free]) so no on-chip transposes are needed; the host pre-transposes the
input shard and pre-tiles the weights into [n_tile][pk, k_tile, jn]
slabs so every DMA is contiguous. Matmuls run in bf16 (host-cast), PSUM
accumulation in fp32.

logsumexp uses the constant bound max=1.0 (normalized rows: sim <= 1),
so no row-max pass is needed: lse = 1/T + log(sum_j exp((S_ij-1)/T)).
The self term is excluded by subtracting exp((S_ii-1)/T) where S_ii is
recomputed locally with bit-identical operands (the gathered block is a
byte-copy of the local features). pos similarities are the diagonals of
the local block-gram with the partner block ((m+4) mod 8).
"""

import os
import sys

for _p in ("/opt/trn_rl_repo",):
    if os.path.isdir(_p) and _p not in sys.path:
        sys.path.append(_p)

import ml_dtypes
import numpy as np

import concourse.bass as bass
import concourse.mybir as mybir
import concourse.tile as tile
from concourse import bacc
from concourse.bass_utils import run_bass_kernel_spmd
from concourse.masks import make_identity

BF16 = ml_dtypes.bfloat16
F32 = mybir.dt.float32
BF = mybir.dt.bfloat16
F8 = mybir.dt.float8e4
FP8 = mybir.dt.np(F8)

B, D, H, E = 4096, 2048, 2048, 128
T = 0.07
SCALE = float(1.0 / T)
NCORES = 8
BS = B // NCORES          # rows per view per core (512)
M = 2 * BS                # local feature rows (1024)
KT = D // 128             # 16 contraction tiles for D/H
NT = H // 128             # 16 output-feature tiles for hidden layers
MT = M // 128             # 8 local row tiles
NG = NCORES * M           # 8192 gathered rows
NCHUNK = NG // 512        # 16 sim free-dim chunks per row tile
SKIP = set(os.environ.get("KERNEL_SKIP", "").split(",")) - {""}


def _build():
    nc = bacc.Bacc(num_devices=NCORES)

    x = nc.dram_tensor("x", [128, KT, M], F8, kind="ExternalInput")
    w0 = nc.dram_tensor("w0", [NT, 128, KT, 128], F8, kind="ExternalInput")
    w1 = nc.dram_tensor("w1", [NT, 128, KT, 128], F8, kind="ExternalInput")
    w2 = nc.dram_tensor("w2", [128, KT, 128], BF, kind="ExternalInput")
    b0 = nc.dram_tensor("b0", [128, NT], F32, kind="ExternalInput")
    b1 = nc.dram_tensor("b1", [128, NT], F32, kind="ExternalInput")
    b2 = nc.dram_tensor("b2", [128, 1], F32, kind="ExternalInput")
    out = nc.dram_tensor("out", [128, MT], F32, kind="ExternalOutput")

    AF = mybir.ActivationFunctionType

    with tile.TileContext(nc) as tc:
        with (
            tc.tile_pool(name="acts", bufs=2) as acts,
            tc.tile_pool(name="wp", bufs=3) as wp,
            tc.tile_pool(name="singles", bufs=1) as singles,
            tc.tile_pool(name="small", bufs=4) as small,
            tc.tile_pool(name="esc", bufs=4) as esc,
            tc.tile_pool(name="pmm", bufs=4, space="PSUM") as pmm,
            tc.tile_pool(name="psmall", bufs=2, space="PSUM") as psmall,
            tc.tile_pool(name="dram", bufs=1, space="DRAM") as dram,
        ):
            # ---- constants ----
            ident = singles.tile([128, 128], F32)
            make_identity(nc, ident)
            b0s = singles.tile([128, NT], F32)
            b1s = singles.tile([128, NT], F32)
            b2s = singles.tile([128, 1], F32)
            nc.sync.dma_start(out=b0s, in_=b0[:, :])
            nc.sync.dma_start(out=b1s, in_=b1[:, :])
            nc.sync.dma_start(out=b2s, in_=b2[:, :])

            # ---- load transposed input activations ----
            a_x = acts.tile([128, KT, M], F8, tag="acts")
            for tk in range(KT):
                nc.sync.dma_start(out=a_x[:, tk, :], in_=x[:, tk, :])

            def mlp_layer(src, dst_tag, wdram, bias_s, func, ntiles,
                          in_dt=BF, out_dt=BF):
                """src: [128, KT, M]; returns [128, ntiles, M] tile."""
                fp8 = in_dt == F8
                kstep = 2 if fp8 else 1
                pmode = mybir.MatmulPerfMode.DoubleRow if fp8 else None
                dst = acts.tile([128, ntiles, M], out_dt, tag=dst_tag)
                for tn in range(ntiles):
                    wsl = wp.tile([128, KT, 128], in_dt, tag="w")
                    nc.sync.dma_start(
                        out=wsl, in_=wdram[tn] if ntiles > 1 else wdram[:, :, :]
                    )
                    for mc in range(2):
                        ps = pmm.tile([128, 512], F32, tag="mm")
                        msl = slice(mc * 512, (mc + 1) * 512)
                        for tk in range(0, KT, kstep):
                            if fp8:
                                nc.tensor.matmul(
                                    ps,
                                    lhsT=wsl[:, tk : tk + 2, :],
                                    rhs=src[:, tk : tk + 2, msl],
                                    start=(tk == 0),
                                    stop=(tk == KT - 2),
                                    perf_mode=pmode,
                                )
                            else:
                                nc.tensor.matmul(
                                    ps,
                                    lhsT=wsl[:, tk, :],
                                    rhs=src[:, tk, msl],
                                    start=(tk == 0),
                                    stop=(tk == KT - 1),
                                )
                        nc.scalar.activation(
                            out=dst[:, tn, msl],
                            in_=ps,
                            func=func,
                            bias=bias_s[:, tn : tn + 1],
                            scale=1.0,
                        )
                return dst

            a_h0 = mlp_layer(a_x, "acts", w0, b0s, AF.Relu, NT, in_dt=F8, out_dt=F8)
            a_h1 = mlp_layer(a_h0, "acts", w1, b1s, AF.Identity, NT, in_dt=F8, out_dt=BF)

            # ---- layer 2 -> eT [128(E), M] fp32 ----
            eT = singles.tile([128, M], F32)
            wsl2 = singles.tile([128, KT, 128], BF)
            nc.sync.dma_start(out=wsl2, in_=w2[:, :, :])
            for mc in range(2):
                ps = pmm.tile([128, 512], F32, tag="mm")
                msl = slice(mc * 512, (mc + 1) * 512)
                for tk in range(KT):
                    nc.tensor.matmul(
                        ps,
                        lhsT=wsl2[:, tk, :],
                        rhs=a_h1[:, tk, msl],
                        start=(tk == 0),
                        stop=(tk == KT - 1),
                    )
                nc.scalar.activation(
                    out=eT[:, msl], in_=ps, func=AF.Identity,
                    bias=b2s[:, 0:1], scale=1.0,
                )

            # ---- normalize columns of eT -> fT (bf16) ----
            ones = singles.tile([128, 128], F32)
            nc.vector.memset(ones, 1.0)
            nbias = singles.tile([128, 1], F32)
            nc.vector.memset(nbias, -SCALE)
            pbias = singles.tile([128, 1], F32)
            nc.vector.memset(pbias, SCALE)
            sq = singles.tile([128, M], F32)
            nc.vector.tensor_mul(sq, eT, eT)
            rnorm = singles.tile([128, M], F32)
            fT = singles.tile([128, M], BF)
            for mc in range(2):
                msl = slice(mc * 512, (mc + 1) * 512)
                ps = pmm.tile([128, 512], F32, tag="mm")
                nc.tensor.matmul(ps, lhsT=ones, rhs=sq[:, msl], start=True, stop=True)
                nc.scalar.activation(
                    out=rnorm[:, msl], in_=ps, func=AF.Sqrt, scale=1.0
                )
                nc.vector.reciprocal(out=rnorm[:, msl], in_=rnorm[:, msl])
                nc.vector.tensor_mul(fT[:, msl], eT[:, msl], rnorm[:, msl])

            # ---- self/pos diagonals from local features (fills gather stall) ----
            dself_all = singles.tile([128, MT], F32)
            dpos_all = singles.tile([128, MT], F32)
            for m in range(MT):
                pm = (m + MT // 2) % MT
                lhs = fT[:, m * 128 : (m + 1) * 128]
                ps_self = psmall.tile([128, 128], F32, tag="ps_small")
                nc.tensor.matmul(
                    ps_self, lhsT=lhs, rhs=fT[:, m * 128 : (m + 1) * 128],
                    start=True, stop=True,
                )
                dsc = small.tile([128, 128], F32, tag="dscratch")
                nc.vector.tensor_mul(dsc, ps_self, ident)
                nc.vector.reduce_sum(
                    dself_all[:, m : m + 1], dsc, axis=mybir.AxisListType.X
                )
                ps_pos = psmall.tile([128, 128], F32, tag="ps_small")
                nc.tensor.matmul(
                    ps_pos, lhsT=lhs, rhs=fT[:, pm * 128 : (pm + 1) * 128],
                    start=True, stop=True,
                )
                dsc2 = small.tile([128, 128], F32, tag="dscratch")
                nc.vector.tensor_mul(dsc2, ps_pos, ident)
                nc.vector.reduce_sum(
                    dpos_all[:, m : m + 1], dsc2, axis=mybir.AxisListType.X
                )

            # ---- all-gather normalized features ----
            cc_in = dram.tile([128, M], BF)
            cc_out = dram.tile([NCORES * 128, M], BF)
            nc.sync.dma_start(out=cc_in, in_=fT)
            if "collective" in SKIP:
                for r in range(NCORES):
                    nc.sync.dma_start(
                        out=cc_out[r * 128 : (r + 1) * 128, :], in_=cc_in[:, :]
                    )
            else:
                nc.gpsimd.collective_compute(
                    "AllGather",
                    mybir.AluOpType.bypass,
                    replica_groups=[list(range(NCORES))],
                    ins=[cc_in.opt()],
                    outs=[cc_out.opt()],
                )
            FT = singles.tile([128, NG], BF)
            for r in range(NCORES):
                nc.sync.dma_start(
                    out=FT[:, r * M : (r + 1) * M],
                    in_=cc_out[r * 128 : (r + 1) * 128, :],
                )

            # ---- sim + exp-sum per local row tile ----
            outv = singles.tile([128, MT], F32)
            stot_all = singles.tile([128, MT], F32)
            if "phase3" in SKIP:
                nc.vector.tensor_copy(outv, fT[:, :MT])
            for m in ([] if "phase3" in SKIP else range(MT)):
                lhs = fT[:, m * 128 : (m + 1) * 128]
                sums = small.tile([128, NCHUNK], F32, tag="sums")
                for c in range(NCHUNK):
                    ps = pmm.tile([128, 512], F32, tag="mm")
                    nc.tensor.matmul(
                        ps, lhsT=lhs, rhs=FT[:, c * 512 : (c + 1) * 512],
                        start=True, stop=True,
                    )
                    escr = esc.tile([128, 512], BF, tag="escr")
                    nc.scalar.activation(
                        out=escr, in_=ps, func=AF.Exp, scale=SCALE, bias=nbias
                    )
                    nc.vector.reduce_sum(
                        sums[:, c : c + 1], escr, axis=mybir.AxisListType.X
                    )
                nc.vector.reduce_sum(
                    stot_all[:, m : m + 1], sums, axis=mybir.AxisListType.X
                )

            # ---- batched epilogue (one ACT table load per function) ----
            if "phase3" not in SKIP:
                eself = small.tile([128, MT], F32, tag="eself")
                nc.scalar.activation(
                    out=eself, in_=dself_all, func=AF.Exp, scale=SCALE, bias=nbias
                )
                sexcl = small.tile([128, MT], F32, tag="sexcl")
                nc.vector.tensor_sub(sexcl, stot_all, eself)
                lsep = small.tile([128, MT], F32, tag="lsep")
                nc.scalar.activation(out=lsep, in_=sexcl, func=AF.Ln, scale=1.0)
                post = small.tile([128, MT], F32, tag="post")
                nc.scalar.activation(
                    out=post, in_=dpos_all, func=AF.Identity, scale=-SCALE, bias=pbias
                )
                nc.vector.tensor_add(outv, lsep, post)

            nc.sync.dma_start(out=out[:, :], in_=outv)

    nc.finalize()
    return nc


_NC_CACHE = None


def _get_nc():
    global _NC_CACHE
    if _NC_CACHE is None:
        _NC_CACHE = _build()
    return _NC_CACHE


def _prep_w(W, ntiles, dt=BF16):
    K = W.shape[0]
    kt = K // 128
    arr = W.reshape(kt, 128, ntiles, 128).transpose(2, 1, 0, 3)
    return np.ascontiguousarray(arr.astype(dt))


def _prep_b(b, ntiles):
    return np.ascontiguousarray(
        np.asarray(b, np.float32).reshape(ntiles, 128).T
    )


def kernel(input1, input2, W0, b0, W1, b1, W2, b2):
    input1 = np.asarray(input1, np.float32)
    input2 = np.asarray(input2, np.float32)
    w0p = _prep_w(np.asarray(W0, np.float32), NT, FP8)
    w1p = _prep_w(np.asarray(W1, np.float32), NT, FP8)
    w2p = _prep_w(np.asarray(W2, np.float32), 1)[0]
    b0p = _prep_b(b0, NT)
    b1p = _prep_b(b1, NT)
    b2p = np.ascontiguousarray(np.asarray(b2, np.float32).reshape(128, 1))

    in_maps = []
    for r in range(NCORES):
        xr = np.concatenate(
            [input1[r * BS : (r + 1) * BS], input2[r * BS : (r + 1) * BS]], axis=0
        )
        xp = np.ascontiguousarray(
            xr.reshape(M, KT, 128).transpose(2, 1, 0).astype(FP8)
        )
        in_maps.append(
            {
                "x": xp, "w0": w0p, "w1": w1p, "w2": w2p,
                "b0": b0p, "b1": b1p, "b2": b2p,
            }
        )

    nc = _get_nc()
    res = run_bass_kernel_spmd(
        nc,
        in_maps,
        core_ids=list(range(NCORES)),
        trace=bool(int(os.environ.get("KERNEL_TRACE", "0"))),
    )
    total = np.float64(0.0)
    for r in range(NCORES):
        total += np.asarray(res.results[r]["out"], np.float64).sum()
    loss = np.float32(total / (2 * B))
    if res.exec_time_ns is not None:
        kernel.last_exec_time_ns = res.exec_time_ns
    return np.asarray(loss, np.float32)


kernel.last_exec_time_ns = None



# revision 9
# speedup vs baseline: 1.1287x; 1.1287x over previous
"""ContrastiveHead loss kernel for 8 Trainium2 NeuronCores (v2).

Strategy (per sharding hint): data-parallel shard B across the 8 cores.
Each core runs the 3-layer MLP for its 2*B/8 = 1024 rows (input1 and
input2 shards stacked), normalizes the [1024, 128] features, all-gathers
the fp8-quantized normalized features across cores, then computes its
local [1024, 8192] block of the similarity matrix and the masked
logsumexp.

v2 changes over v1 (323 us):
 - fp8 all-gather, split into two halves pipelined with the sim phase
   (the v1 single bf16 gather left ~60 us of dead time).
 - sim-phase exp: ScalarE activation with accum_out (kills the VectorE
   TENSOR_REDUCE, 92 us in v1), FD=2048 groups (4 PSUM banks / read),
   and ~1/3 of the groups offloaded to VectorE via a Schraudolph
   exp approximation (affine -> int32 convert -> bitcast fp32), with the
   self-diagonal exclusion recomputed through whichever engine owned the
   self group (per-core `sel` mask input blends the two recomputes).
 - layer 2 in fp8 DoubleRow (was bf16).
 - norm reciprocal via reciprocal_approx_fast (was 6.7 us of DVE divide).
 - ACT table prewarm for Sqrt (during L0) and Exp (during the gather).

logsumexp uses the constant bound max=1.0 (normalized rows: sim <= 1),
so no row-max pass is needed: lse = 1/T + log(sum_j exp((S_ij-1)/T)).
The self term is excluded by subtracting exp((S_ii-1)/T) where S_ii is
recomputed locally with bit-identical operands. pos similarities are the
diagonals of the local block-gram with the partner block ((m+4) mod 8).
"""

import os
import sys

for _p in ("/opt/trn_rl_repo",):
    if os.path.isdir(_p) and _p not in sys.path:
        sys.path.append(_p)

import ml_dtypes
import numpy as np

import concourse.bass as bass
import concourse.mybir as mybir
import concourse.tile as tile
from concourse import bacc
from concourse.bass_utils import run_bass_kernel_spmd
from concourse.masks import make_identity

BF16 = ml_dtypes.bfloat16
F32 = mybir.dt.float32
I32 = mybir.dt.int32
BF = mybir.dt.bfloat16
F8 = mybir.dt.float8e4
FP8 = mybir.dt.np(F8)

B, D, H, E = 4096, 2048, 2048, 128
T = 0.07
SCALE = float(1.0 / T)
NCORES = 8
BS = B // NCORES          # rows per view per core (512)
M = 2 * BS                # local feature rows (1024)
KT = D // 128             # 16 contraction tiles for D/H
NT = H // 128             # 16 output-feature tiles for hidden layers
MT = M // 128             # 8 local row tiles
NG = NCORES * M           # 8192 gathered rows
SKIP = set(os.environ.get("KERNEL_SKIP", "").split(",")) - {""}
# Debug bisection switches: "exp" = v1-style sim exp (no accum_out, no
# Schraudolph, FD=512 ACT reads); "sch" = disable only the Schraudolph
# DVE offload; "acc" = disable only accum_out (ACT exp + DVE reduce).
SAFE = set(os.environ.get("KERNEL_V2_SAFE", "").split(",")) - {""}

# Schraudolph exp constants: exp(SCALE*v - SCALE) ~= bitcast(int32(A*v + BB))
LOG2E = 1.4426950408889634
SCH_C = 0.05774
SCH_A = float(np.float32((1 << 23) * LOG2E * SCALE))
SCH_B = float(np.float32((1 << 23) * (127.0 - SCH_C - SCALE * LOG2E)))

# Sim-phase engine assignment: per half h, set of (m, j) 2048-col groups
# handled by the VectorE Schraudolph path (rest go to ScalarE exp).
DVE_SET = {
    0: {(0, 1), (1, 1), (3, 1), (5, 1), (7, 1)},
    1: {(0, 1), (1, 1), (2, 1), (4, 1), (6, 1)},
}


def _dve_active():
    return not ({"exp", "sch"} & SAFE)


def _build():
    nc = bacc.Bacc(num_devices=NCORES)

    x = nc.dram_tensor("x", [128, KT, M], F8, kind="ExternalInput")
    w0 = nc.dram_tensor("w0", [NT, 128, KT, 128], F8, kind="ExternalInput")
    w1 = nc.dram_tensor("w1", [NT, 128, KT, 128], F8, kind="ExternalInput")
    w2 = nc.dram_tensor("w2", [128, KT, 128], F8, kind="ExternalInput")
    b0 = nc.dram_tensor("b0", [128, NT], F32, kind="ExternalInput")
    b1 = nc.dram_tensor("b1", [128, NT], F32, kind="ExternalInput")
    b2 = nc.dram_tensor("b2", [128, 1], F32, kind="ExternalInput")
    seli = nc.dram_tensor("sel", [128, MT], F32, kind="ExternalInput")
    out = nc.dram_tensor("out", [128, MT], F32, kind="ExternalOutput")

    AF = mybir.ActivationFunctionType
    ALU = mybir.AluOpType

    with tile.TileContext(nc) as tc:
        with (
            tc.tile_pool(name="acts", bufs=2) as acts,
            tc.tile_pool(name="wp", bufs=3) as wp,
            tc.tile_pool(name="singles", bufs=1) as singles,
            tc.tile_pool(name="small", bufs=4) as small,
            tc.tile_pool(name="esc", bufs=2) as esc,
            tc.tile_pool(name="pmm", bufs=2, space="PSUM") as pmm,
            tc.tile_pool(name="dram", bufs=1, space="DRAM") as dram,
        ):
            # ---- load transposed input activations (first: feeds L0) ----
            a_x = acts.tile([128, KT, M], F8, tag="acts")
            for tk in range(KT):
                nc.sync.dma_start(out=a_x[:, tk, :], in_=x[:, tk, :])

            b0s = singles.tile([128, NT], F32)
            b1s = singles.tile([128, NT], F32)
            b2s = singles.tile([128, 1], F32)
            sels = singles.tile([128, MT], F32)
            nc.sync.dma_start(out=b0s, in_=b0[:, :])
            nc.sync.dma_start(out=b1s, in_=b1[:, :])
            nc.sync.dma_start(out=b2s, in_=b2[:, :])
            nc.sync.dma_start(out=sels, in_=seli[:, :])

            ident = singles.tile([128, 128], F32)
            make_identity(nc, ident)
            ones = singles.tile([128, 128], F32)
            nc.vector.memset(ones, 1.0)
            nbias = singles.tile([128, 1], F32)
            nc.vector.memset(nbias, -SCALE)
            pbias = singles.tile([128, 1], F32)
            nc.vector.memset(pbias, SCALE)

            def mlp_layer(src, dst_tag, wdram, bias_s, func, out_dt):
                """src: [128, KT, M] f8; returns [128, NT, M] tile (fp8 DR)."""
                dst = acts.tile([128, NT, M], out_dt, tag=dst_tag)
                for tn in range(NT):
                    wsl = wp.tile([128, KT, 128], F8, tag="w")
                    half = KT // 2
                    nc.sync.dma_start(out=wsl[:, :half, :], in_=wdram[tn, :, :half, :])
                    nc.sync.dma_start(out=wsl[:, half:, :], in_=wdram[tn, :, half:, :])
                    ps = pmm.tile([128, 2048], F32, tag="mm")
                    for mc in range(2):
                        msl = slice(mc * 512, (mc + 1) * 512)
                        for tk in range(0, KT, 2):
                            nc.tensor.matmul(
                                ps[:, mc * 512 : (mc + 1) * 512],
                                lhsT=wsl[:, tk : tk + 2, :],
                                rhs=src[:, tk : tk + 2, msl],
                                start=(tk == 0),
                                stop=(tk == KT - 2),
                                perf_mode=mybir.MatmulPerfMode.DoubleRow,
                            )
                    nc.scalar.activation(
                        out=dst[:, tn, :],
                        in_=ps[:, 0:1024],
                        func=func,
                        bias=bias_s[:, tn : tn + 1],
                        scale=1.0,
                    )
                return dst

            a_h0 = mlp_layer(a_x, "acts", w0, b0s, AF.Relu, F8)
            # prewarm the Sqrt activation table while the PE is busy in L0
            warm = singles.tile([128, 1], F32)
            nc.scalar.activation(out=warm, in_=b2s, func=AF.Sqrt, scale=0.0, bias=1.0)
            a_h1 = mlp_layer(a_h0, "acts", w1, b1s, AF.Identity, F8)

            # ---- layer 2 (fp8 DoubleRow) -> eT [128(E), M] fp32 ----
            wsl2 = singles.tile([128, KT, 128], F8)
            nc.sync.dma_start(out=wsl2, in_=w2[:, :, :])
            eT = singles.tile([128, M], F32)
            fT = singles.tile([128, M], BF)
            f8T = singles.tile([128, M], F8)
            sq = singles.tile([128, M], F32)
            rno = singles.tile([128, M], F32)
            rrec = singles.tile([128, M], F32)
            ps2 = pmm.tile([128, 2048], F32, tag="mm")
            cc_in = [None, None]
            cc_out = [None, None]
            for h in range(2):
                cc_in[h] = dram.tile([128, BS], F8, name=f"cc_in{h}")
                cc_out[h] = dram.tile([NCORES * 128, BS], F8, name=f"cc_out{h}")

            for mc in range(2):
                msl = slice(mc * 512, (mc + 1) * 512)
                for tk in range(0, KT, 2):
                    nc.tensor.matmul(
                        ps2[:, mc * 512 : (mc + 1) * 512],
                        lhsT=wsl2[:, tk : tk + 2, :],
                        rhs=a_h1[:, tk : tk + 2, msl],
                        start=(tk == 0),
                        stop=(tk == KT - 2),
                        perf_mode=mybir.MatmulPerfMode.DoubleRow,
                    )
                nc.scalar.activation(
                    out=eT[:, msl], in_=ps2[:, mc * 512 : (mc + 1) * 512],
                    func=AF.Identity, bias=b2s[:, 0:1], scale=1.0,
                )
                # ---- normalize columns of this half, quantize to fp8 ----
                nc.vector.tensor_mul(sq[:, msl], eT[:, msl], eT[:, msl])
                psn = pmm.tile([128, 2048], F32, tag="mm")
                nc.tensor.matmul(
                    psn[:, 0:512], lhsT=ones, rhs=sq[:, msl], start=True, stop=True
                )
                nc.scalar.activation(
                    out=rno[:, msl], in_=psn[:, 0:512], func=AF.Sqrt, scale=1.0
                )
                nc.vector.reciprocal_approx_fast(
                    out=rrec[:, msl], in_=rno[:, msl]
                )
                nc.vector.tensor_mul(fT[:, msl], eT[:, msl], rrec[:, msl])
                nc.scalar.activation(out=f8T[:, msl], in_=fT[:, msl], func=AF.Copy)
                # ---- kick this half's all-gather ----
                nc.sync.dma_start(out=cc_in[mc], in_=f8T[:, msl])
                if "collective" in SKIP:
                    for r in range(NCORES):
                        nc.sync.dma_start(
                            out=cc_out[mc][r * 128 : (r + 1) * 128, :],
                            in_=cc_in[mc][:, :],
                        )
                else:
                    nc.gpsimd.collective_compute(
                        "AllGather",
                        ALU.bypass,
                        replica_groups=[list(range(NCORES))],
                        ins=[cc_in[mc].opt()],
                        outs=[cc_out[mc].opt()],
                    )

            # prewarm the Exp table during the gather
            warm2 = singles.tile([128, 1], F32)
            nc.scalar.activation(out=warm2, in_=b2s, func=AF.Exp, scale=0.0, bias=0.0)

            # ---- self/pos diagonals from local fp8 features ----
            dself = singles.tile([128, MT], F32)
            dpos = singles.tile([128, MT], F32)
            for m in range(MT):
                pm = (m + MT // 2) % MT
                lhs = f8T[:, m * 128 : (m + 1) * 128]
                ps_s = pmm.tile([128, 2048], F32, tag="mm")
                nc.tensor.matmul(
                    ps_s[:, 0:128], lhsT=lhs, rhs=f8T[:, m * 128 : (m + 1) * 128],
                    start=True, stop=True,
                )
                dsc = small.tile([128, 128], F32, tag="dscratch")
                nc.vector.tensor_mul(dsc, ps_s[:, 0:128], ident)
                nc.vector.reduce_sum(
                    dself[:, m : m + 1], dsc, axis=mybir.AxisListType.X
                )
                ps_p = pmm.tile([128, 2048], F32, tag="mm")
                nc.tensor.matmul(
                    ps_p[:, 0:128], lhsT=lhs, rhs=f8T[:, pm * 128 : (pm + 1) * 128],
                    start=True, stop=True,
                )
                dsc2 = small.tile([128, 128], F32, tag="dscratch")
                nc.vector.tensor_mul(dsc2, ps_p[:, 0:128], ident)
                nc.vector.reduce_sum(
                    dpos[:, m : m + 1], dsc2, axis=mybir.AxisListType.X
                )

            # ---- gathered features: FT col i = h*4096 + r2*512 + k ----
            FT = singles.tile([128, NG], F8)
            for h in range(2):
                for r in range(NCORES):
                    nc.sync.dma_start(
                        out=FT[:, h * 4096 + r * 512 : h * 4096 + (r + 1) * 512],
                        in_=cc_out[h][r * 128 : (r + 1) * 128, :],
                    )

            # ---- sim + exp-sum per local row tile, split ACT/DVE ----
            sA = singles.tile([128, 4 * MT], F32)
            outv = singles.tile([128, MT], F32)
            if "phase3" in SKIP:
                nc.vector.tensor_copy(outv, dself)
            for h in ([] if "phase3" in SKIP else range(2)):
                for m in range(MT):
                    lhs = f8T[:, m * 128 : (m + 1) * 128]
                    for j in range(2):
                        ps = pmm.tile([128, 2048], F32, tag="mm")
                        base = h * 4096 + j * 2048
                        for q in range(4):
                            nc.tensor.matmul(
                                ps[:, q * 512 : (q + 1) * 512],
                                lhsT=lhs,
                                rhs=FT[:, base + q * 512 : base + (q + 1) * 512],
                                start=True,
                                stop=True,
                            )
                        col = (2 * h + j) * MT + m
                        use_dve = (m, j) in DVE_SET[h] and not (
                            "exp" in SAFE or "sch" in SAFE
                        )
                        if use_dve:
                            i32t = esc.tile([128, 2048], I32, tag="i32")
                            nc.vector.tensor_scalar(
                                out=i32t, in0=ps, scalar1=SCH_A, scalar2=SCH_B,
                                op0=ALU.mult, op1=ALU.add,
                            )
                            nc.vector.reduce_sum(
                                sA[:, col : col + 1],
                                i32t.bitcast(F32),
                                axis=mybir.AxisListType.X,
                            )
                        elif "exp" in SAFE or "acc" in SAFE:
                            qs = small.tile([128, 4], F32, tag="qsum")
                            for q in range(4):
                                escr = esc.tile([128, 2048], BF, tag="escr")
                                nc.scalar.activation(
                                    out=escr[:, 0:512],
                                    in_=ps[:, q * 512 : (q + 1) * 512],
                                    func=AF.Exp, scale=SCALE, bias=nbias,
                                )
                                nc.vector.reduce_sum(
                                    qs[:, q : q + 1], escr[:, 0:512],
                                    axis=mybir.AxisListType.X,
                                )
                            nc.vector.reduce_sum(
                                sA[:, col : col + 1], qs, axis=mybir.AxisListType.X
                            )
                        else:
                            escr = esc.tile([128, 2048], F8, tag="escr")
                            nc.scalar.activation(
                                out=escr, in_=ps, func=AF.Exp, scale=SCALE,
                                bias=nbias, accum_out=sA[:, col : col + 1],
                            )

            # ---- epilogue ----
            if "phase3" not in SKIP:
                s01 = small.tile([128, MT], F32, tag="ep")
                s23 = small.tile([128, MT], F32, tag="ep")
                stot = small.tile([128, MT], F32, tag="ep")
                nc.vector.tensor_add(s01, sA[:, 0:MT], sA[:, MT : 2 * MT])
                nc.vector.tensor_add(s23, sA[:, 2 * MT : 3 * MT], sA[:, 3 * MT :])
                nc.vector.tensor_add(stot, s01, s23)
                eselfA = small.tile([128, MT], F32, tag="ep")
                nc.scalar.activation(
                    out=eselfA, in_=dself, func=AF.Exp, scale=SCALE, bias=nbias
                )
                if _dve_active():
                    es_i = small.tile([128, MT], I32, tag="ep")
                    nc.vector.tensor_scalar(
                        out=es_i, in0=dself, scalar1=SCH_A, scalar2=SCH_B,
                        op0=ALU.mult, op1=ALU.add,
                    )
                    dcorr = small.tile([128, MT], F32, tag="ep")
                    nc.vector.tensor_sub(dcorr, es_i.bitcast(F32), eselfA)
                    dcorr2 = small.tile([128, MT], F32, tag="ep")
                    nc.vector.tensor_mul(dcorr2, dcorr, sels)
                    eself = small.tile([128, MT], F32, tag="ep")
                    nc.vector.tensor_add(eself, eselfA, dcorr2)
                else:
                    eself = eselfA
                sexcl = small.tile([128, MT], F32, tag="ep")
                nc.vector.tensor_sub(sexcl, stot, eself)
                lsep = small.tile([128, MT], F32, tag="ep")
                nc.scalar.activation(out=lsep, in_=sexcl, func=AF.Ln, scale=1.0)
                post = small.tile([128, MT], F32, tag="ep")
                nc.scalar.activation(
                    out=post, in_=dpos, func=AF.Identity, scale=-SCALE, bias=pbias
                )
                nc.vector.tensor_add(outv, lsep, post)

            nc.sync.dma_start(out=out[:, :], in_=outv)

    nc.finalize()
    return nc


_NC_CACHE = None


def _get_nc():
    global _NC_CACHE
    if _NC_CACHE is None:
        _NC_CACHE = _build()
    return _NC_CACHE


def _prep_w(W, ntiles, dt=BF16):
    K = W.shape[0]
    kt = K // 128
    arr = W.reshape(kt, 128, ntiles, 128).transpose(2, 1, 0, 3)
    return np.ascontiguousarray(arr.astype(dt))


def _prep_b(b, ntiles):
    return np.ascontiguousarray(
        np.asarray(b, np.float32).reshape(ntiles, 128).T
    )


def kernel(input1, input2, W0, b0, W1, b1, W2, b2):
    input1 = np.asarray(input1, np.float32)
    input2 = np.asarray(input2, np.float32)
    w0p = _prep_w(np.asarray(W0, np.float32), NT, FP8)
    w1p = _prep_w(np.asarray(W1, np.float32), NT, FP8)
    w2p = _prep_w(np.asarray(W2, np.float32), 1, FP8)[0]
    b0p = _prep_b(b0, NT)
    b1p = _prep_b(b1, NT)
    b2p = np.ascontiguousarray(np.asarray(b2, np.float32).reshape(128, 1))

    in_maps = []
    for r in range(NCORES):
        xr = np.concatenate(
            [input1[r * BS : (r + 1) * BS], input2[r * BS : (r + 1) * BS]], axis=0
        )
        xp = np.ascontiguousarray(
            xr.reshape(M, KT, 128).transpose(2, 1, 0).astype(FP8)
        )
        selv = np.zeros((128, MT), np.float32)
        for m in range(MT):
            if (m, r // 4) in DVE_SET[m // 4]:
                selv[:, m] = 1.0
        in_maps.append(
            {
                "x": xp, "w0": w0p, "w1": w1p, "w2": w2p,
                "b0": b0p, "b1": b1p, "b2": b2p, "sel": selv,
            }
        )

    nc = _get_nc()
    res = run_bass_kernel_spmd(
        nc,
        in_maps,
        core_ids=list(range(NCORES)),
        trace=bool(int(os.environ.get("KERNEL_TRACE", "0"))),
    )
    total = np.float64(0.0)
    for r in range(NCORES):
        total += np.asarray(res.results[r]["out"], np.float64).sum()
    loss = np.float32(total / (2 * B))
    if res.exec_time_ns is not None:
        kernel.last_exec_time_ns = res.exec_time_ns
    return np.asarray(loss, np.float32)


kernel.last_exec_time_ns = None


# revision 11
# speedup vs baseline: 1.2657x; 1.1214x over previous
"""ContrastiveHead loss kernel for 8 Trainium2 NeuronCores (v3).

Strategy (per sharding hint): data-parallel shard B across the 8 cores.
Each core runs the 3-layer MLP for its 2*B/8 = 1024 rows (input1 and
input2 shards stacked), normalizes the [1024, 128] features, all-gathers
the fp8-quantized normalized features across cores, then computes its
local [1024, 8192] block of the similarity matrix and the masked
logsumexp.

v3 structure: the local batch is processed in two row-halves A/B pushed
through the whole MLP+norm pipeline, so half A's all-gather overlaps
half B's MLP on the PE, and the A-rows x A-cols sim block overlaps too.
A tiny warm-up AllGather at kernel start absorbs the first-collective
setup cost. Sim-phase exp runs on ScalarE (activation with accum_out,
FD=2048 groups spanning 4 PSUM banks) with ~1/3 of groups offloaded to
VectorE via a Schraudolph exp approximation (affine -> int32 convert ->
bitcast); the self-diagonal exclusion is recomputed through whichever
engine owned the self group (per-core `sel` input blends the two).

logsumexp uses the constant bound max=1.0 (normalized rows: sim <= 1):
lse = 1/T + log(sum_j exp((S_ij-1)/T)), self term excluded by
subtracting exp((S_ii-1)/T) with S_ii recomputed from bit-identical
operands. pos similarities are diagonals of the local block-gram with
the partner block ((m+4) mod 8).
"""

import os
import sys

for _p in ("/opt/trn_rl_repo",):
    if os.path.isdir(_p) and _p not in sys.path:
        sys.path.append(_p)

import ml_dtypes
import numpy as np

import concourse.bass as bass
import concourse.mybir as mybir
import concourse.tile as tile
from concourse import bacc
from concourse.bass_utils import run_bass_kernel_spmd
from concourse.masks import make_identity

BF16 = ml_dtypes.bfloat16
F32 = mybir.dt.float32
I32 = mybir.dt.int32
BF = mybir.dt.bfloat16
F8 = mybir.dt.float8e4
FP8 = mybir.dt.np(F8)

B, D, H, E = 4096, 2048, 2048, 128
T = 0.07
SCALE = float(1.0 / T)
NCORES = 8
BS = B // NCORES          # rows per view per core (512)
M = 2 * BS                # local feature rows (1024)
KT = D // 128             # 16 contraction tiles for D/H
NT = H // 128             # 16 output-feature tiles for hidden layers
MT = M // 128             # 8 local row tiles
NG = NCORES * M           # 8192 gathered rows
SKIP = set(os.environ.get("KERNEL_SKIP", "").split(",")) - {""}
SAFE = set(os.environ.get("KERNEL_V2_SAFE", "").split(",")) - {""}

# Schraudolph exp constants: exp(SCALE*v - SCALE) ~= bitcast(int32(A*v + B))
LOG2E = 1.4426950408889634
SCH_C = 0.05774
SCH_A = float(np.float32((1 << 23) * LOG2E * SCALE))
SCH_B = float(np.float32((1 << 23) * (127.0 - SCH_C - SCALE * LOG2E)))

# Sim-phase engine assignment: per col-half h, set of (m, j) 2048-col groups
# handled by the VectorE Schraudolph path (rest go to ScalarE exp).
DVE_SET = {
    0: {(0, 1), (1, 1), (3, 1), (5, 1), (7, 1)},
    1: {(0, 1), (1, 1), (2, 1), (4, 1), (6, 1)},
}


def _dve_active():
    return not ({"exp", "sch"} & SAFE)


def _build():
    nc = bacc.Bacc(num_devices=NCORES)

    x = nc.dram_tensor("x", [128, KT, M], F8, kind="ExternalInput")
    w0 = nc.dram_tensor("w0", [NT, 128, KT, 128], F8, kind="ExternalInput")
    w1 = nc.dram_tensor("w1", [NT, 128, KT, 128], F8, kind="ExternalInput")
    w2 = nc.dram_tensor("w2", [128, KT, 128], F8, kind="ExternalInput")
    b0 = nc.dram_tensor("b0", [128, NT], F32, kind="ExternalInput")
    b1 = nc.dram_tensor("b1", [128, NT], F32, kind="ExternalInput")
    b2 = nc.dram_tensor("b2", [128, 1], F32, kind="ExternalInput")
    seli = nc.dram_tensor("sel", [128, MT], F32, kind="ExternalInput")
    out = nc.dram_tensor("out", [128, MT], F32, kind="ExternalOutput")

    AF = mybir.ActivationFunctionType
    ALU = mybir.AluOpType
    groups = [list(range(NCORES))]

    with tile.TileContext(nc) as tc:
        with (
            tc.tile_pool(name="acts", bufs=3) as acts,
            tc.tile_pool(name="wp", bufs=3) as wp,
            tc.tile_pool(name="singles", bufs=1) as singles,
            tc.tile_pool(name="small", bufs=4) as small,
            tc.tile_pool(name="esc", bufs=2) as esc,
            tc.tile_pool(name="pmm", bufs=2, space="PSUM") as pmm,
            tc.tile_pool(name="dram", bufs=1, space="DRAM") as dram,
        ):
            # ---- input activations: two big DMAs; L0 can start after #1 ----
            a_x = acts.tile([128, KT, M], F8, tag="acts")
            nc.sync.dma_start(out=a_x[:, : KT // 2, :], in_=x[:, : KT // 2, :])
            b0s = singles.tile([128, NT], F32)
            nc.sync.dma_start(out=b0s, in_=b0[:, :])
            nc.sync.dma_start(out=a_x[:, KT // 2 :, :], in_=x[:, KT // 2 :, :])

            # ---- warm-up collective: absorb first-cc channel setup ----
            ccw_s = small.tile([128, 8], F8, tag="ccw")
            nc.vector.memset(ccw_s, 0.0)
            ccw_in = dram.tile([128, 8], F8, name="ccw_in")
            ccw_out = dram.tile([NCORES * 128, 8], F8, name="ccw_out")
            nc.sync.dma_start(out=ccw_in, in_=ccw_s)
            if "collective" not in SKIP:
                nc.gpsimd.collective_compute(
                    "AllGather", ALU.bypass, replica_groups=groups,
                    ins=[ccw_in.opt()], outs=[ccw_out.opt()],
                )

            a_h0 = acts.tile([128, NT, M], F8, tag="acts")
            a_h1 = acts.tile([128, NT, M], F8, tag="acts")
            eT = singles.tile([128, M], F32)
            fT = singles.tile([128, M], BF)
            f8T = singles.tile([128, M], F8)
            sq = singles.tile([128, M], F32)
            rno = singles.tile([128, M], F32)
            rrec = singles.tile([128, M], F32)
            FT = singles.tile([128, NG], F8)
            sA = singles.tile([128, 4 * MT], F32)
            dself = singles.tile([128, MT], F32)
            dpos = singles.tile([128, MT], F32)
            outv = singles.tile([128, MT], F32)
            wsl2 = singles.tile([128, KT, 128], F8)
            cc_in = [None, None]
            cc_out = [None, None]
            for h in range(2):
                cc_in[h] = dram.tile([128, BS], F8, name=f"cc_in{h}")
                cc_out[h] = dram.tile([NCORES * 128, BS], F8, name=f"cc_out{h}")

            def mlp_layer(src, dst, wdram, bias_s, func, rs):
                """One fp8-DoubleRow layer for row slice rs (512 rows)."""
                for tn in range(NT):
                    wsl = wp.tile([128, KT, 128], F8, tag="w")
                    nc.sync.dma_start(out=wsl, in_=wdram[tn])
                    ps = pmm.tile([128, 2048], F32, tag="mm")
                    for tk in range(0, KT, 2):
                        nc.tensor.matmul(
                            ps[:, 0:512],
                            lhsT=wsl[:, tk : tk + 2, :],
                            rhs=src[:, tk : tk + 2, rs],
                            start=(tk == 0),
                            stop=(tk == KT - 2),
                            perf_mode=mybir.MatmulPerfMode.DoubleRow,
                        )
                    nc.scalar.activation(
                        out=dst[:, tn, rs], in_=ps[:, 0:512], func=func,
                        bias=bias_s[:, tn : tn + 1], scale=1.0,
                    )

            warm = singles.tile([128, 1], F32)
            warm2 = singles.tile([128, 1], F32)
            ident = singles.tile([128, 128], F32)
            ones = singles.tile([128, 128], F32)
            nbias = singles.tile([128, 1], F32)
            pbias = singles.tile([128, 1], F32)
            b1s = singles.tile([128, NT], F32)
            b2s = singles.tile([128, 1], F32)
            sels = singles.tile([128, MT], F32)

            def mlp_half(half):
                rs = slice(half * 512, (half + 1) * 512)
                mlp_layer(a_x, a_h0, w0, b0s, AF.Relu, rs)
                if half == 0:
                    # constants + prewarm Sqrt table while the PE is busy
                    nc.sync.dma_start(out=b1s, in_=b1[:, :])
                    nc.sync.dma_start(out=b2s, in_=b2[:, :])
                    nc.sync.dma_start(out=sels, in_=seli[:, :])
                    nc.sync.dma_start(out=wsl2, in_=w2[:, :, :])
                    nc.vector.memset(ones, 1.0)
                    nc.vector.memset(nbias, -SCALE)
                    nc.vector.memset(pbias, SCALE)
                    nc.scalar.activation(
                        out=warm, in_=b0s[:, 0:1], func=AF.Sqrt, scale=0.0, bias=1.0
                    )
                mlp_layer(a_h0, a_h1, w1, b1s, AF.Identity, rs)
                # layer 2 (fp8 DR, 8 matmuls) + normalize + quantize + gather
                ps2 = pmm.tile([128, 2048], F32, tag="mm")
                for tk in range(0, KT, 2):
                    nc.tensor.matmul(
                        ps2[:, 0:512],
                        lhsT=wsl2[:, tk : tk + 2, :],
                        rhs=a_h1[:, tk : tk + 2, rs],
                        start=(tk == 0),
                        stop=(tk == KT - 2),
                        perf_mode=mybir.MatmulPerfMode.DoubleRow,
                    )
                nc.scalar.activation(
                    out=eT[:, rs], in_=ps2[:, 0:512], func=AF.Identity,
                    bias=b2s[:, 0:1], scale=1.0,
                )
                nc.vector.tensor_mul(sq[:, rs], eT[:, rs], eT[:, rs])
                psn = pmm.tile([128, 2048], F32, tag="mm")
                nc.tensor.matmul(
                    psn[:, 0:512], lhsT=ones, rhs=sq[:, rs], start=True, stop=True
                )
                nc.scalar.activation(
                    out=rno[:, rs], in_=psn[:, 0:512], func=AF.Sqrt, scale=1.0
                )
                nc.vector.reciprocal_approx_fast(out=rrec[:, rs], in_=rno[:, rs])
                nc.vector.tensor_mul(fT[:, rs], eT[:, rs], rrec[:, rs])
                nc.scalar.activation(out=f8T[:, rs], in_=fT[:, rs], func=AF.Copy)
                nc.sync.dma_start(out=cc_in[half], in_=f8T[:, rs])
                if "collective" in SKIP:
                    for r in range(NCORES):
                        nc.sync.dma_start(
                            out=cc_out[half][r * 128 : (r + 1) * 128, :],
                            in_=cc_in[half][:, :],
                        )
                else:
                    nc.gpsimd.collective_compute(
                        "AllGather", ALU.bypass, replica_groups=groups,
                        ins=[cc_in[half].opt()], outs=[cc_out[half].opt()],
                    )

            def sim_group(m, h, j):
                """One [128 rows x 2048 gathered cols] exp-sum group."""
                lhs = f8T[:, m * 128 : (m + 1) * 128]
                ps = pmm.tile([128, 2048], F32, tag="mm")
                base = h * 4096 + j * 2048
                for q in range(4):
                    nc.tensor.matmul(
                        ps[:, q * 512 : (q + 1) * 512],
                        lhsT=lhs,
                        rhs=FT[:, base + q * 512 : base + (q + 1) * 512],
                        start=True,
                        stop=True,
                    )
                col = (2 * h + j) * MT + m
                use_dve = (m, j) in DVE_SET[h] and _dve_active()
                if use_dve:
                    i32t = esc.tile([128, 2048], I32, tag="i32")
                    nc.vector.tensor_scalar(
                        out=i32t, in0=ps, scalar1=SCH_A, scalar2=SCH_B,
                        op0=ALU.mult, op1=ALU.add,
                    )
                    nc.vector.reduce_sum(
                        sA[:, col : col + 1], i32t.bitcast(F32),
                        axis=mybir.AxisListType.X,
                    )
                elif "exp" in SAFE or "acc" in SAFE:
                    qs = small.tile([128, 4], F32, tag="qsum")
                    for q in range(4):
                        escr = esc.tile([128, 2048], BF, tag="escr")
                        nc.scalar.activation(
                            out=escr[:, 0:512], in_=ps[:, q * 512 : (q + 1) * 512],
                            func=AF.Exp, scale=SCALE, bias=nbias,
                        )
                        nc.vector.reduce_sum(
                            qs[:, q : q + 1], escr[:, 0:512],
                            axis=mybir.AxisListType.X,
                        )
                    nc.vector.reduce_sum(
                        sA[:, col : col + 1], qs, axis=mybir.AxisListType.X
                    )
                else:
                    escr = esc.tile([128, 2048], F8, tag="escr")
                    nc.scalar.activation(
                        out=escr, in_=ps, func=AF.Exp, scale=SCALE,
                        bias=nbias, accum_out=sA[:, col : col + 1],
                    )

            def load_ft(h):
                for r in range(NCORES):
                    nc.sync.dma_start(
                        out=FT[:, h * 4096 + r * 512 : h * 4096 + (r + 1) * 512],
                        in_=cc_out[h][r * 128 : (r + 1) * 128, :],
                    )

            # ---- phase A: rows 0-511 through MLP+norm, gather A ----
            mlp_half(0)
            # prewarm Exp table (runs right after norm A on the ACT queue)
            nc.scalar.activation(
                out=warm2, in_=b0s[:, 0:1], func=AF.Exp, scale=0.0, bias=0.0
            )

            # ---- phase B rows 512-1023; A-cols sim interleaved ----
            rsB = slice(512, 1024)
            mlp_layer(a_x, a_h0, w0, b0s, AF.Relu, rsB)
            mlp_layer(a_h0, a_h1, w1, b1s, AF.Identity, rsB)
            # sim: A rows x A cols (gather A landed during L0-B/L1-B);
            # consumers fill the gather-B wait window
            load_ft(0)
            phase3 = "phase3" not in SKIP
            if phase3:
                for m in range(4):
                    for j in range(2):
                        sim_group(m, 0, j)
            # finish B: L2 + norm + gather B (inlined via mlp_half tail)
            ps2 = pmm.tile([128, 2048], F32, tag="mm")
            for tk in range(0, KT, 2):
                nc.tensor.matmul(
                    ps2[:, 0:512],
                    lhsT=wsl2[:, tk : tk + 2, :],
                    rhs=a_h1[:, tk : tk + 2, rsB],
                    start=(tk == 0),
                    stop=(tk == KT - 2),
                    perf_mode=mybir.MatmulPerfMode.DoubleRow,
                )
            nc.scalar.activation(
                out=eT[:, rsB], in_=ps2[:, 0:512], func=AF.Identity,
                bias=b2s[:, 0:1], scale=1.0,
            )
            nc.vector.tensor_mul(sq[:, rsB], eT[:, rsB], eT[:, rsB])
            psn = pmm.tile([128, 2048], F32, tag="mm")
            nc.tensor.matmul(
                psn[:, 0:512], lhsT=ones, rhs=sq[:, rsB], start=True, stop=True
            )
            nc.scalar.activation(
                out=rno[:, rsB], in_=psn[:, 0:512], func=AF.Sqrt, scale=1.0
            )
            nc.vector.reciprocal_approx_fast(out=rrec[:, rsB], in_=rno[:, rsB])
            nc.vector.tensor_mul(fT[:, rsB], eT[:, rsB], rrec[:, rsB])
            nc.scalar.activation(out=f8T[:, rsB], in_=fT[:, rsB], func=AF.Copy)
            nc.sync.dma_start(out=cc_in[1], in_=f8T[:, rsB])
            if "collective" in SKIP:
                for r in range(NCORES):
                    nc.sync.dma_start(
                        out=cc_out[1][r * 128 : (r + 1) * 128, :],
                        in_=cc_in[1][:, :],
                    )
            else:
                nc.gpsimd.collective_compute(
                    "AllGather", ALU.bypass, replica_groups=groups,
                    ins=[cc_in[1].opt()], outs=[cc_out[1].opt()],
                )

            # ---- diagonals (need both halves of f8T) ----
            make_identity(nc, ident)
            for m in range(MT):
                pm = (m + MT // 2) % MT
                lhs = f8T[:, m * 128 : (m + 1) * 128]
                ps_s = pmm.tile([128, 2048], F32, tag="mm")
                nc.tensor.matmul(
                    ps_s[:, 0:128], lhsT=lhs, rhs=f8T[:, m * 128 : (m + 1) * 128],
                    start=True, stop=True,
                )
                dsc = small.tile([128, 128], F32, tag="dscratch")
                nc.vector.tensor_mul(dsc, ps_s[:, 0:128], ident)
                nc.vector.reduce_sum(
                    dself[:, m : m + 1], dsc, axis=mybir.AxisListType.X
                )
                ps_p = pmm.tile([128, 2048], F32, tag="mm")
                nc.tensor.matmul(
                    ps_p[:, 0:128], lhsT=lhs, rhs=f8T[:, pm * 128 : (pm + 1) * 128],
                    start=True, stop=True,
                )
                dsc2 = small.tile([128, 128], F32, tag="dscratch")
                nc.vector.tensor_mul(dsc2, ps_p[:, 0:128], ident)
                nc.vector.reduce_sum(
                    dpos[:, m : m + 1], dsc2, axis=mybir.AxisListType.X
                )

            # ---- remaining sim groups: B rows x A cols first (no cc wait),
            #      then all x B cols once gather B lands ----
            if phase3:
                for m in range(4, 8):
                    for j in range(2):
                        sim_group(m, 0, j)
                load_ft(1)
                for m in range(MT):
                    for j in range(2):
                        sim_group(m, 1, j)
            else:
                nc.vector.tensor_copy(outv, dself)

            # ---- epilogue ----
            if phase3:
                s01 = small.tile([128, MT], F32, tag="ep")
                s23 = small.tile([128, MT], F32, tag="ep")
                stot = small.tile([128, MT], F32, tag="ep")
                nc.vector.tensor_add(s01, sA[:, 0:MT], sA[:, MT : 2 * MT])
                nc.vector.tensor_add(s23, sA[:, 2 * MT : 3 * MT], sA[:, 3 * MT :])
                nc.vector.tensor_add(stot, s01, s23)
                eselfA = small.tile([128, MT], F32, tag="ep")
                nc.scalar.activation(
                    out=eselfA, in_=dself, func=AF.Exp, scale=SCALE, bias=nbias
                )
                if _dve_active():
                    es_i = small.tile([128, MT], I32, tag="ep")
                    nc.vector.tensor_scalar(
                        out=es_i, in0=dself, scalar1=SCH_A, scalar2=SCH_B,
                        op0=ALU.mult, op1=ALU.add,
                    )
                    dcorr = small.tile([128, MT], F32, tag="ep")
                    nc.vector.tensor_sub(dcorr, es_i.bitcast(F32), eselfA)
                    dcorr2 = small.tile([128, MT], F32, tag="ep")
                    nc.vector.tensor_mul(dcorr2, dcorr, sels)
                    eself = small.tile([128, MT], F32, tag="ep")
                    nc.vector.tensor_add(eself, eselfA, dcorr2)
                else:
                    eself = eselfA
                sexcl = small.tile([128, MT], F32, tag="ep")
                nc.vector.tensor_sub(sexcl, stot, eself)
                lsep = small.tile([128, MT], F32, tag="ep")
                nc.scalar.activation(out=lsep, in_=sexcl, func=AF.Ln, scale=1.0)
                post = small.tile([128, MT], F32, tag="ep")
                nc.scalar.activation(
                    out=post, in_=dpos, func=AF.Identity, scale=-SCALE, bias=pbias
                )
                nc.vector.tensor_add(outv, lsep, post)

            nc.sync.dma_start(out=out[:, :], in_=outv)

    nc.finalize()
    return nc


_NC_CACHE = None


def _get_nc():
    global _NC_CACHE
    if _NC_CACHE is None:
        _NC_CACHE = _build()
    return _NC_CACHE


def _prep_w(W, ntiles, dt=BF16):
    K = W.shape[0]
    kt = K // 128
    arr = W.reshape(kt, 128, ntiles, 128).transpose(2, 1, 0, 3)
    return np.ascontiguousarray(arr.astype(dt))


def _prep_b(b, ntiles):
    return np.ascontiguousarray(
        np.asarray(b, np.float32).reshape(ntiles, 128).T
    )


def kernel(input1, input2, W0, b0, W1, b1, W2, b2):
    input1 = np.asarray(input1, np.float32)
    input2 = np.asarray(input2, np.float32)
    w0p = _prep_w(np.asarray(W0, np.float32), NT, FP8)
    w1p = _prep_w(np.asarray(W1, np.float32), NT, FP8)
    w2p = _prep_w(np.asarray(W2, np.float32), 1, FP8)[0]
    b0p = _prep_b(b0, NT)
    b1p = _prep_b(b1, NT)
    b2p = np.ascontiguousarray(np.asarray(b2, np.float32).reshape(128, 1))

    in_maps = []
    for r in range(NCORES):
        xr = np.concatenate(
            [input1[r * BS : (r + 1) * BS], input2[r * BS : (r + 1) * BS]], axis=0
        )
        xp = np.ascontiguousarray(
            xr.reshape(M, KT, 128).transpose(2, 1, 0).astype(FP8)
        )
        selv = np.zeros((128, MT), np.float32)
        for m in range(MT):
            if (m, r // 4) in DVE_SET[m // 4]:
                selv[:, m] = 1.0
        in_maps.append(
            {
                "x": xp, "w0": w0p, "w1": w1p, "w2": w2p,
                "b0": b0p, "b1": b1p, "b2": b2p, "sel": selv,
            }
        )

    nc = _get_nc()
    res = run_bass_kernel_spmd(
        nc,
        in_maps,
        core_ids=list(range(NCORES)),
        trace=bool(int(os.environ.get("KERNEL_TRACE", "0"))),
    )
    total = np.float64(0.0)
    for r in range(NCORES):
        total += np.asarray(res.results[r]["out"], np.float64).sum()
    loss = np.float32(total / (2 * B))
    if res.exec_time_ns is not None:
        kernel.last_exec_time_ns = res.exec_time_ns
    return np.asarray(loss, np.float32)


kernel.last_exec_time_ns = None
